# revision 69
# baseline (speedup 1.0000x reference)
"""Trainium2 Bass kernel: transformer encoder layer (B=4, S=2048, D=1024, H=16, FF=4096).

Sharding (8 NeuronCores, no collectives): core c handles batch b=c//2 and
query-token half r=c%2 (1024 query rows). K/V are recomputed per core over the
batch's full 2048-token sequence (~12% duplicated FLOPs, zero communication).

Device layout: all activations are kept feature-on-partition ("transposed",
[d, tokens]) so every projection is matmul(lhsT=weight_natural, rhs=act_T).
Attention computes scores^T [k, q] per head (softmax denominators come from a
ones-column appended to V — row 64 of the AV accumulation), so no on-device
transposes are needed anywhere. The host passes x already transposed with the
core's query tokens first (attention is permutation-invariant over k; the
src_mask is all-ones).

Numerics: matmul operands and the residual stream are bf16 (each residual
store rounds at ~0.2% rms — well inside the 2e-2 budget); PSUM accumulation,
LN statistics, and the DMA'd output stay fp32. LN sums read the bf16 stream
directly (bf16 ones column); sum-of-squares goes through an f32r Square on
ScalarE. Biases are exact (b_v folds into b_o on the host). Softmax skips
the max-subtraction: scores/8 are O(6), exp() is far from overflow.

Schedule: post-attention work is software-pipelined by hand. w_o(chunk 0)
interleaves with the second half of attention; w_o(chunk 1) and the LN1 sum
matmuls run while LN1 stats/normalize (VectorE) drain; each later LN hides
under the next chunk's FFN matmuls. ScalarE (idle outside exp) applies
matmul biases and LN squares; GpSimd does no compute (its elementwise ops
are ~3x slower) — it only hosts DMA queues. QKV runs m-major with the token
tiles innermost so each loaded weight column block is reused across 2-4
matmuls. w2 streams per-128-column chunk from a host-pre-tiled layout (8 KB
contiguous per partition); the final output is DMA'd per 128-feature chunk.
"""

import numpy as np
import ml_dtypes

import concourse.bass as bass
import concourse.tile as tile
from concourse import bacc
from concourse import mybir
from concourse.bass_utils import run_bass_kernel_spmd

P = 128
D = 1024          # d_model
S = 2048          # kv sequence length per core (one full batch)
TQ = 1024         # query tokens per core
H = 16            # heads
DK = 64           # head dim
FF = 4096         # ffn dim
DO = D // P       # 8  d_model chunks
KC = S // P       # 16 kv-token chunks
FO = FF // P      # 32 ffn chunks
NF = 512          # matmul free-dim tile
EPS = 1e-5

BF16 = mybir.dt.bfloat16
F32 = mybir.dt.float32
F32R = mybir.dt.float32r
FP8 = mybir.dt.float8e4
DR = mybir.MatmulPerfMode.DoubleRow
AF = mybir.ActivationFunctionType
ALU = mybir.AluOpType


def build():
    nc = bacc.Bacc("TRN2", target_bir_lowering=False, debug=False, num_devices=8)

    # all big operands arrive pre-tiled from the host so every DMA moves
    # >=4 KB contiguous per partition (1 KB-segment patterns measured ~65 GB/s)
    xt = nc.dram_tensor("xt", [P, DO, S], BF16, kind="ExternalInput").ap()
    wq = nc.dram_tensor("wq", [2, P, DO, NF], BF16, kind="ExternalInput").ap()
    wk = nc.dram_tensor("wk", [2, P, DO, NF], BF16, kind="ExternalInput").ap()
    wv = nc.dram_tensor("wv", [2, P, DO, NF], BF16, kind="ExternalInput").ap()
    wo = nc.dram_tensor("wo", [P, DO, D], BF16, kind="ExternalInput").ap()
    w1 = nc.dram_tensor("w1", [DO, P, DO, NF], BF16, kind="ExternalInput").ap()
    # w2 pre-tiled on host: [DO(m), 128(p=k-chunk lane), FO(kc), 128(m cols)]
    # so each 1 MB m-chunk is 8 KB contiguous per partition.
    w2p = nc.dram_tensor("w2p", [DO, P, FO, P], BF16, kind="ExternalInput").ap()
    bq = nc.dram_tensor("bq", [D], F32, kind="ExternalInput").ap()
    bk = nc.dram_tensor("bk", [D], F32, kind="ExternalInput").ap()
    bo = nc.dram_tensor("bo", [D], F32, kind="ExternalInput").ap()  # b_o + b_v@w_o
    b1v = nc.dram_tensor("b1", [FF], F32, kind="ExternalInput").ap()
    b2v = nc.dram_tensor("b2", [D], F32, kind="ExternalInput").ap()
    gb1 = nc.dram_tensor("gb1", [2, D], BF16, kind="ExternalInput").ap()
    gb2 = nc.dram_tensor("gb2", [2, D], BF16, kind="ExternalInput").ap()
    onesr = nc.dram_tensor("onesr", [P], F32R, kind="ExternalInput").ap()
    yt = nc.dram_tensor("yt", [D, TQ], F32, kind="ExternalOutput").ap()

    yt3 = yt.rearrange("(o p) t -> p o t", p=P)

    with tile.TileContext(nc) as tc:
        with (
            tc.tile_pool(name="persist", bufs=1) as persist,
            tc.tile_pool(name="lns", bufs=1) as lns,
            tc.tile_pool(name="work", bufs=2) as work,
            tc.tile_pool(name="psum", bufs=4, space="PSUM") as psum,
            tc.tile_pool(name="pssc", bufs=2, space="PSUM") as pssc,
        ):
            def load_vec(ap, n_chunks, name, eng):
                t = persist.tile([P, n_chunks], F32, tag=name)
                eng.dma_start(out=t[:], in_=ap.rearrange("(o p) -> p o", p=P))
                return t

            ones_col = persist.tile([P, 1], F32R, tag="ones_col")
            ones_col_bf = persist.tile([P, 1], BF16, tag="ones_col_bf")
            nc.vector.memset(ones_col_bf[:], 1.0)
            ones_row_dk = persist.tile([1, DK], BF16, tag="ones_row_dk")
            nc.vector.memset(ones_row_dk[:], 1.0)
            eps_sb = persist.tile([P, 1], F32, tag="eps")
            nc.vector.memset(eps_sb[:], EPS)
            eshift_sb = persist.tile([P, 1], F32, tag="eshift")
            nc.vector.memset(eshift_sb[:], -1.5)

            # ---------- layernorm pieces (feature dim on partitions) ----------
            def ln_sums(h_sl):
                """h_sl: [P, DO, NF] bf16. Returns (ps_s, ps_q) PSUM [1, NF]."""
                ps_s = psum.tile([P, NF], F32, tag="mm")
                ps_q = psum.tile([P, NF], F32, tag="mm")
                for o in range(DO):
                    nc.tensor.matmul(ps_s[0:1, :], lhsT=ones_col_bf,
                                     rhs=h_sl[:, o, :],
                                     start=(o == 0), stop=(o == DO - 1))
                    sq = work.tile([P, NF], F32R, tag="sq")
                    nc.scalar.activation(out=sq[:], in_=h_sl[:, o, :],
                                         func=AF.Square)
                    nc.tensor.matmul(ps_q[0:1, :], lhsT=ones_col, rhs=sq[:],
                                     start=(o == 0), stop=(o == DO - 1))
                return ps_s, ps_q

            def ln_stats(ps_s, ps_q, name):
                """Small per-token stats chain. Returns (rstd_bf [1,NF],
                nm [2,NF]) where nm row0 = -mean*rstd and row1 = ones."""
                mean = lns.tile([1, NF], F32, tag="ln_mean", name=f"mean_{name}")
                msq = lns.tile([1, NF], F32, tag="ln_msq", name=f"msq_{name}")
                nc.vector.tensor_scalar_mul(mean[:], ps_s[0:1, :], 1.0 / D)
                nc.vector.tensor_scalar_mul(msq[:], ps_q[0:1, :], 1.0 / D)
                var = lns.tile([1, NF], F32, tag="ln_var", name=f"var_{name}")
                nc.vector.tensor_mul(var[:], mean[:], mean[:])
                nc.vector.tensor_sub(var[:], msq[:], var[:])
                nc.scalar.activation(out=var[:], in_=var[:], func=AF.Sqrt,
                                     bias=eps_sb[0:1])
                rstd = lns.tile([1, NF], F32, tag="ln_rstd", name=f"rstd_{name}")
                nc.vector.reciprocal_approx_fast(out=rstd[:], in_=var[:])
                negms = msq  # msq dead; reuse
                nc.vector.tensor_mul(negms[:], mean[:], rstd[:])
                nc.vector.tensor_scalar_mul(negms[:], negms[:], -1.0)
                rstd_bf = lns.tile([1, NF], BF16, tag="ln_rstd_bf",
                                   name=f"rstdb_{name}")
                nc.vector.tensor_copy(rstd_bf[:], rstd[:])
                nm = lns.tile([2, NF], BF16, tag="ln_nm", name=f"nm_{name}")
                nc.vector.memset(nm[:], 1.0)  # row1 stays ones
                nc.vector.tensor_copy(nm[0:1, :], negms[:])
                return rstd_bf, nm

            def ln_bcast_o(o, rstd_bf, nm, gb_sb, name):
                """Per-o-chunk broadcast with gamma/beta folded in:
                row0 slot: g_o (x) rstd; row1 slot: g_o (x) negms + be_o (x) 1.
                bf16 K<=2 matmuls; rounding is ~0.2% rms — in budget."""
                ps = pssc.tile([P, 2, NF], F32, tag="sc2",
                               name=f"lnb_{name}_{o}")
                nc.tensor.matmul(ps[:, 0, :],
                                 lhsT=gb_sb[0:1, bass.ts(o, P)],
                                 rhs=rstd_bf[:], start=True, stop=True)
                nc.tensor.matmul(ps[:, 1, :],
                                 lhsT=gb_sb[0:2, bass.ts(o, P)],
                                 rhs=nm[:], start=True, stop=True)
                return ps

            def ln_norm_o(h_sl, o, ps, yt_out=None, dma_eng=None):
                if yt_out is None:
                    nc.vector.tensor_mul(h_sl[:, o, :], h_sl[:, o, :],
                                         ps[:, 0, :])
                    nc.vector.tensor_add(h_sl[:, o, :], h_sl[:, o, :],
                                         ps[:, 1, :])
                else:
                    yto = work.tile([P, NF], F32, tag="yto")
                    nc.vector.tensor_mul(yto[:], h_sl[:, o, :], ps[:, 0, :])
                    nc.vector.tensor_add(yto[:], yto[:], ps[:, 1, :])
                    dma_eng.dma_start(out=yt_out[:, o, :], in_=yto[:])

            with (
                tc.tile_pool(name="h1", bufs=1, side="right") as h1p,
                tc.tile_pool(name="wop", bufs=1, side="right") as wop,
            ):
                # h1f is preloaded with x (the residual); w_o accumulates
                # into it through a small ScalarE-biased temp
                h1f = h1p.tile([P, DO, TQ], BF16, tag="h1f")
                wo_sb = wop.tile([P, DO, D], BF16, tag="wo")

                def emit_wo_m(qn, m):
                    sl = bass.ts(qn, NF)
                    ps = psum.tile([P, NF], F32, tag="mm")
                    for kc in range(DO):
                        nc.tensor.matmul(
                            ps[:], lhsT=wo_sb[:, kc, bass.ts(m, P)],
                            rhs=ctx[:, kc, sl],
                            start=(kc == 0), stop=(kc == DO - 1))
                    wot = work.tile([P, NF], BF16, tag="wot")
                    nc.scalar.activation(
                        out=wot[:], in_=ps[:], func=AF.Identity,
                        bias=bo_sb[:, m:m + 1])
                    nc.vector.tensor_add(
                        h1f[:, m, sl], h1f[:, m, sl], wot[:])

                with tc.tile_pool(name="kqv", bufs=1) as kqvp:
                    kT = kqvp.tile([P, DO, S], BF16, tag="kT")
                    qT = kqvp.tile([P, DO, TQ], BF16, tag="qT")
                    # V and the exp'd scores are fp8e4m3: their quantization
                    # noise averages out across ~2k attention weights per
                    # token, and it enables DoubleRow AV matmuls (2 k-chunks
                    # per instruction)
                    vaug = kqvp.tile([P, KC, H * 65], FP8, tag="vaug")
                    vaug_h = vaug.rearrange("p t (h w) -> p t h w", w=65)
                    ctx = kqvp.tile([P, DO, TQ], BF16, tag="ctx")

                    # ---- phase 1: QKV projections ----
                    with (
                        tc.tile_pool(name="xtp", bufs=1) as xtp,
                        tc.tile_pool(name="wst", bufs=3) as wst,
                    ):
                        # startup: the gpsimd engine comes up ~20us late, so
                        # everything needed early rides the sync/scalar
                        # queues; gpsimd gets only far-future prefetches.
                        # Weights arrive in half-1024-column tiles so the
                        # m-major matmuls start after ~1 MB instead of 2 MB;
                        # xt streams per-128-feature chunk, matching the
                        # K-accumulation consumption order.
                        wk_h = [wst.tile([P, DO, NF], BF16, tag="w",
                                         name=f"wk_{h}") for h in range(2)]
                        nc.sync.dma_start(out=wk_h[0][:], in_=wk[0])
                        xt_sb = xtp.tile([P, DO, S], BF16, tag="xt")
                        # single-descriptor 1 MB quarters (per-partition
                        # contiguous), matching the K accumulation order
                        nc.scalar.dma_start(out=xt_sb[:, 0:2, :],
                                            in_=xt[:, 0:2, :])
                        nc.scalar.dma_start(out=xt_sb[:, 2:4, :],
                                            in_=xt[:, 2:4, :])
                        nc.sync.dma_start(out=xt_sb[:, 4:6, :],
                                          in_=xt[:, 4:6, :])
                        nc.sync.dma_start(out=wk_h[1][:], in_=wk[1])
                        nc.sync.dma_start(out=xt_sb[:, 6:8, :],
                                          in_=xt[:, 6:8, :])
                        for t in range(KC):
                            nc.vector.memset(vaug_h[:, t, :, 64:65], 1.0)
                        bk_sb = load_vec(bk, DO, "bk", nc.scalar)
                        wq_h = [wst.tile([P, DO, NF], BF16, tag="w",
                                         name=f"wq_{h}") for h in range(2)]
                        for h in range(2):
                            nc.sync.dma_start(out=wq_h[h][:], in_=wq[h])
                        bq_sb = load_vec(bq, DO, "bq", nc.sync)
                        # late-use loads on the slow-starting gpsimd queue
                        nc.gpsimd.dma_start(out=ones_col[:], in_=onesr[:, None])
                        bo_sb = load_vec(bo, DO, "bo", nc.gpsimd)
                        b2_sb = load_vec(b2v, DO, "b2", nc.gpsimd)
                        gb1_sb = persist.tile([2, D], BF16, tag="gb1")
                        nc.gpsimd.dma_start(out=gb1_sb[:], in_=gb1)
                        gb2_sb = persist.tile([2, D], BF16, tag="gb2")
                        nc.gpsimd.dma_start(out=gb2_sb[:], in_=gb2)
                        b1_sb = load_vec(b1v, FO, "b1", nc.gpsimd)
                        # prefetches for the post-attention pipeline (idle
                        # queues during QKV/attention); h1f starts as x
                        nc.gpsimd.dma_start(out=wo_sb[:], in_=wo)
                        nc.gpsimd.dma_start(out=h1f[:], in_=xt[:, :, 0:TQ])

                        # K^T/Q^T m-major, token tiles innermost: each weight
                        # column block is lhsT for 2 matmuls. K runs as two
                        # passes over token halves so the first pass only
                        # waits on the first 2 MB of xt.
                        def proj_T(w_h, out_sb, bias_sb, ns, tag):
                            for m in range(DO):
                                hh, mh = m // 4, m % 4
                                pss = [psum.tile([P, NF], F32, tag="mm",
                                                 name=f"pj_{tag}_{m}_{n}")
                                       for n in ns]
                                for kc in range(DO):
                                    for i, n in enumerate(ns):
                                        nc.tensor.matmul(
                                            pss[i][:],
                                            lhsT=w_h[hh][:, kc, bass.ts(mh, P)],
                                            rhs=xt_sb[:, kc, bass.ts(n, NF)],
                                            start=(kc == 0), stop=(kc == DO - 1))
                                for i, n in enumerate(ns):
                                    nc.vector.tensor_scalar(
                                        out=out_sb[:, m, bass.ts(n, NF)],
                                        in0=pss[i][:],
                                        scalar1=bias_sb[:, m:m + 1],
                                        scalar2=None, op0=ALU.add)

                        proj_T(wk_h, kT, bk_sb, [0, 1], "ka")
                        proj_T(wk_h, kT, bk_sb, [2, 3], "kb")
                        proj_T(wq_h, qT, bq_sb, [0, 1], "q")

                        # V in natural [token, d] layout, heads padded to 65
                        # cols (col 64 = ones -> softmax denominator); x token
                        # blocks are lhsT, reused across the 2 output tiles
                        wv_h = [wst.tile([P, DO, NF], BF16, tag="w",
                                         name=f"wv_{h}") for h in range(2)]
                        for h in range(2):
                            nc.sync.dma_start(out=wv_h[h][:], in_=wv[h])
                        for t in range(KC):
                            pss = [psum.tile([P, NF], F32, tag="mm",
                                             name=f"pv_{t}_{n}")
                                   for n in range(2)]
                            for kc in range(DO):
                                for n in range(2):
                                    nc.tensor.matmul(
                                        pss[n][:],
                                        lhsT=xt_sb[:, kc, bass.ts(t, P)],
                                        rhs=wv_h[n][:, kc, :],
                                        start=(kc == 0), stop=(kc == DO - 1))
                            for n in range(2):
                                nc.vector.tensor_copy(
                                    out=vaug_h[:, t, 8 * n:8 * n + 8, 0:64],
                                    in_=pss[n].rearrange("p (h w) -> p h w",
                                                         w=DK))

                    # ---- phase 2: attention ----
                    # software-pipelined: emit scores(pair i+1) before AV(pair
                    # i) so PE keeps running while ACT computes the exps. The
                    # w_o matmuls for query chunk 0 slot into the back half.
                    with tc.tile_pool(name="es", bufs=4) as esp:
                        KH = KC // 2  # kv chunks per half
                        units = [(qn, j, half) for qn in range(TQ // NF)
                                 for j in range(H // 2) for half in range(2)]

                        def alloc_es(i):
                            qn, j, half = units[i]
                            # per-head tiles: exp output stays contiguous
                            # (strided fp8 writes measurably slow ACT down)
                            return [esp.tile([P, KH, NF], FP8, tag="es",
                                             name=f"es_{qn}_{j}_{half}_{i2}")
                                    for i2 in range(2)]

                        def emit_scores_g(i, kc2, es01):
                            qn, j, half = units[i]
                            qsl = bass.ts(qn, NF)
                            # two 2-bank PSUM tiles (one per head), exp'd
                            # separately: keeps a second group in flight so
                            # the PE never waits a full exp latency
                            pss = [pssc.tile([P, 2, NF], F32, tag="sc2",
                                             name=f"sc2_{kc2}_{i2}")
                                   for i2 in range(2)]
                            for sub in range(2):
                                kc = half * KH + 2 * kc2 + sub
                                for idx in range(2):
                                    off = idx * DK
                                    nc.tensor.matmul(
                                        pss[idx][:, sub, :],
                                        lhsT=kT[off:off + DK, j, bass.ts(kc, P)],
                                        rhs=qT[off:off + DK, j, qsl],
                                        start=True, stop=True)
                            for idx in range(2):
                                # constant shift keeps exp() under fp8e4m3
                                # max (448); softmax is shift-invariant (the
                                # ones-column denominator shifts too)
                                nc.scalar.activation(
                                    out=es01[idx][:, 2 * kc2:2 * kc2 + 2, :],
                                    in_=pss[idx][:], func=AF.Exp,
                                    scale=0.125, bias=eshift_sb[:, 0:1])

                        def emit_av_part(i, part, es01, pcs):
                            qn, j, half = units[i]
                            for idx in range(2):
                                h = 2 * j + idx
                                for kl in (2 * part, 2 * part + 1):
                                    kc = half * KH + kl
                                    nc.tensor.matmul(
                                        pcs[idx][0:DK + 1, :],
                                        lhsT=vaug[:, kc, h * 65:(h + 1) * 65],
                                        rhs=es01[idx][:, kl, :],
                                        start=(kc == 0), stop=(kc == KC - 1))

                        def emit_av_tail(i, pcs):
                            qn, j, half = units[i]
                            if half == 0:
                                return
                            qsl = bass.ts(qn, NF)
                            for idx in range(2):
                                pc = pcs[idx]
                                den = lns.tile([1, NF], F32, tag="den")
                                nc.vector.tensor_copy(den[:], pc[DK:DK + 1, :])
                                rec = lns.tile([1, NF], F32, tag="rec")
                                nc.vector.reciprocal_approx_fast(
                                    out=rec[:], in_=den[:])
                                recb = esp.tile([1, NF], BF16, tag="recb")
                                nc.vector.tensor_copy(recb[:], rec[:])
                                ps_b = psum.tile([P, NF], F32, tag="mm")
                                nc.tensor.matmul(
                                    ps_b[0:DK, :], lhsT=ones_row_dk,
                                    rhs=recb[:], start=True, stop=True)
                                bc = esp.tile([DK, NF], BF16, tag="bc")
                                nc.vector.tensor_copy(bc[:], ps_b[0:DK, :])
                                nc.vector.tensor_mul(
                                    ctx[idx * DK:(idx + 1) * DK, j, qsl],
                                    pc[0:DK, :], bc[:])

                        # zipper: the PE queue is in-order, so score groups
                        # (which stall on exp draining PSUM) alternate with
                        # always-ready AV chunks of the previous unit
                        pcs = None
                        prev = alloc_es(0)
                        for g in range(KH // 2):
                            emit_scores_g(0, g, prev)
                        for i in range(len(units)):
                            qn, j, half = units[i]
                            cur = prev
                            if half == 0:
                                pcs = [psum.tile([P, NF], F32, tag="mm",
                                                 name=f"pc_{qn}_{j}_{i2}")
                                       for i2 in range(2)]
                            if i + 1 < len(units):
                                prev = alloc_es(i + 1)
                                for g in range(KH // 2):
                                    emit_scores_g(i + 1, g, prev)
                                    emit_av_part(i, g, cur, pcs)
                            else:
                                for g in range(KH // 2):
                                    emit_av_part(i, g, cur, pcs)
                            emit_av_tail(i, pcs)
                            # ctx for chunk 0 completes at unit 15: slide its
                            # w_o into the second half of attention
                            if i >= 16 and i % 2 == 1:
                                emit_wo_m(0, (i - 16) // 2)

                    # still inside the kqv pool (ctx is alive): LN1(0) sums,
                    # then w_o(1) m-chunks pace the LN1(0) stats + normalize
                    sl0, sl1 = bass.ts(0, NF), bass.ts(1, NF)
                    s0 = ln_sums(h1f[:, :, sl0])
                    emit_wo_m(1, 0)
                    emit_wo_m(1, 1)
                    st0 = ln_stats(*s0, name="ln1q0")
                    for o in range(DO):
                        ps = ln_bcast_o(o, *st0, gb1_sb, "l1q0")
                        ln_norm_o(h1f[:, :, sl0], o, ps)
                        if 2 <= o < DO:
                            emit_wo_m(1, o)
                    s1 = ln_sums(h1f[:, :, sl1])
                    st1 = ln_stats(*s1, name="ln1q1")

                # ---- phases 3+4, hand-pipelined across the 2 query chunks ----
                dq = [nc.sync, nc.scalar]
                with (
                    tc.tile_pool(name="aTp", bufs=2) as atp,
                    tc.tile_pool(name="wf1", bufs=2) as wf1p,
                    tc.tile_pool(name="wf2", bufs=2) as wf2p,
                ):
                    aT = [atp.tile([P, FO, NF], BF16, tag="aT", name=f"aT_{qn}")
                          for qn in range(2)]

                    def emit_ffn1_c(qn, c):
                        sl = bass.ts(qn, NF)
                        # 1 MB chunks with 1 KB contiguous rows; smaller
                        # slices DMA far below line rate
                        w1_sb = wf1p.tile([P, DO, NF], BF16, tag="w1")
                        nc.sync.dma_start(out=w1_sb[:], in_=w1[c])
                        for u in range(4):
                            mf = 4 * c + u
                            ps = psum.tile([P, NF], F32, tag="mm")
                            for kc in range(DO):
                                nc.tensor.matmul(
                                    ps[:], lhsT=w1_sb[:, kc, bass.ts(u, P)],
                                    rhs=h1f[:, kc, sl],
                                    start=(kc == 0), stop=(kc == DO - 1))
                            # bias+relu split between ScalarE and VectorE
                            if u % 2 == 0:
                                nc.scalar.activation(
                                    out=aT[qn][:, mf, :], in_=ps[:],
                                    func=AF.Relu, bias=b1_sb[:, mf:mf + 1])
                            else:
                                nc.vector.tensor_scalar(
                                    out=aT[qn][:, mf, :], in0=ps[:],
                                    scalar1=b1_sb[:, mf:mf + 1], scalar2=0.0,
                                    op0=ALU.add, op1=ALU.max)

                    def emit_ffn2_m(qn, m):
                        sl = bass.ts(qn, NF)
                        w2_sb = wf2p.tile([P, FO, P], BF16, tag="w2",
                                          name=f"w2_{qn}_{m}")
                        nc.gpsimd.dma_start(out=w2_sb[:], in_=w2p[m])
                        ps = psum.tile([P, NF], F32, tag="mm")
                        for kc in range(FO):
                            nc.tensor.matmul(
                                ps[:], lhsT=w2_sb[:, kc, :], rhs=aT[qn][:, kc, :],
                                start=(kc == 0), stop=(kc == FO - 1))
                        ep = work.tile([P, NF], BF16, tag="ep")
                        nc.scalar.activation(
                            out=ep[:], in_=ps[:], func=AF.Identity,
                            bias=b2_sb[:, m:m + 1])
                        nc.vector.tensor_add(h1f[:, m, sl], h1f[:, m, sl], ep[:])

                    # normalize both chunks; FFN1(0) fills in behind chunk 0
                    emit_ffn1_c(0, 0)
                    emit_ffn1_c(0, 1)
                    for o in range(DO):
                        ps = ln_bcast_o(o, *st1, gb1_sb, "l1q1")
                        ln_norm_o(h1f[:, :, sl1], o, ps)
                    for c in range(2, 8):
                        emit_ffn1_c(0, c)
                    # FFN2(0); then LN2(0) hides under FFN1(1)
                    for m in range(DO):
                        emit_ffn2_m(0, m)
                    s2 = ln_sums(h1f[:, :, sl0])
                    emit_ffn1_c(1, 0)
                    emit_ffn1_c(1, 1)
                    st2 = ln_stats(*s2, name="ln2q0")
                    for c in range(2, 8):
                        emit_ffn1_c(1, c)
                    for o in range(DO):
                        ps = ln_bcast_o(o, *st2, gb2_sb, "l2q0")
                        ln_norm_o(h1f[:, :, sl0], o, ps,
                                  yt_out=yt3[:, :, sl0], dma_eng=dq[o % 2])
                    # FFN2(1) + LN2(1) tail: the LN sums chase the FFN2
                    # m-chunks so only the stats chain remains at the end
                    ps_s3 = psum.tile([P, NF], F32, tag="mm")
                    ps_q3 = psum.tile([P, NF], F32, tag="mm")
                    for m in range(DO):
                        emit_ffn2_m(1, m)
                        nc.tensor.matmul(ps_s3[0:1, :], lhsT=ones_col_bf,
                                         rhs=h1f[:, m, sl1],
                                         start=(m == 0), stop=(m == DO - 1))
                        sq = work.tile([P, NF], F32R, tag="sq")
                        nc.scalar.activation(out=sq[:], in_=h1f[:, m, sl1],
                                             func=AF.Square)
                        nc.tensor.matmul(ps_q3[0:1, :], lhsT=ones_col, rhs=sq[:],
                                         start=(m == 0), stop=(m == DO - 1))
                    st3 = ln_stats(ps_s3, ps_q3, name="ln2q1")
                    for o in range(DO):
                        ps = ln_bcast_o(o, *st3, gb2_sb, "l2q1")
                        ln_norm_o(h1f[:, :, sl1], o, ps,
                                  yt_out=yt3[:, :, sl1], dma_eng=dq[o % 2])

    nc.compile()
    return nc


_CACHE = {}


def _compiled():
    if "nc" not in _CACHE:
        _CACHE["nc"] = build()
    return _CACHE["nc"]


def make_in_maps(x, w_q, b_q, w_k, b_k, w_v, b_v, w_o, b_o,
                 w1, b1, w2, b2, g1, be1, g2, be2):
    bf = ml_dtypes.bfloat16
    x = np.asarray(x, np.float32)
    f32 = lambda a: np.ascontiguousarray(np.asarray(a, np.float32))

    w_o32 = f32(w_o)
    w2_32 = f32(w2)

    # device-friendly tilings: every DMA reads >=4 KB contiguous/partition
    def tile_proj(w):  # [D, D] -> [2, 128, 8, 512]: w_t[h,p,o,n]=w[o*128+p, 512h+n]
        return np.ascontiguousarray(
            f32(w).reshape(DO, P, 2, NF).transpose(2, 1, 0, 3)).astype(bf)

    # w2p[m, p, kc, c] = w2[kc*128 + p, m*128 + c]
    w2p = np.ascontiguousarray(
        w2_32.reshape(FO, P, DO, P).transpose(2, 1, 0, 3)).astype(bf)
    # w1t[c, p, o, n] = w1[o*128 + p, c*512 + n]
    w1t = np.ascontiguousarray(
        f32(w1).reshape(DO, P, DO, NF).transpose(2, 1, 0, 3)).astype(bf)
    # wo_t[p, o, m] = wo[o*128 + p, m]
    wo_t = np.ascontiguousarray(w_o32.reshape(DO, P, D).transpose(1, 0, 2)
                                ).astype(bf)
    shared = {
        "wq": tile_proj(w_q), "wk": tile_proj(w_k), "wv": tile_proj(w_v),
        "wo": wo_t, "w1": w1t, "w2p": w2p,
        "bq": f32(b_q), "bk": f32(b_k),
        "bo": f32(b_o) + f32(b_v) @ w_o32,
        "b1": f32(b1), "b2": f32(b2),
        "gb1": np.stack([f32(g1), f32(be1)]).astype(bf),
        "gb2": np.stack([f32(g2), f32(be2)]).astype(bf),
        "onesr": np.ones((P,), np.float32),
    }
    in_maps = []
    for c in range(8):
        b, r = c // 2, c % 2
        xb = x[b]
        xc = np.concatenate([xb[r * TQ:(r + 1) * TQ], xb[(1 - r) * TQ:(2 - r) * TQ]],
                            axis=0)
        # xt_t[p, o, t] = x[t, o*128 + p]
        xt_t = np.ascontiguousarray(
            xc.T.reshape(DO, P, S).transpose(1, 0, 2)).astype(bf)
        m = dict(shared)
        m["xt"] = xt_t
        in_maps.append(m)
    return in_maps


def assemble_out(results):
    out = np.empty((4, 2048, 1024), np.float32)
    for c in range(8):
        b, r = c // 2, c % 2
        out[b, r * TQ:(r + 1) * TQ] = results[c]["yt"].T
    return out


def kernel(x, src_mask, w_q, b_q, w_k, b_k, w_v, b_v, w_o, b_o,
           w1, b1, w2, b2, g1, be1, g2, be2):
    in_maps = make_in_maps(x, w_q, b_q, w_k, b_k, w_v, b_v, w_o, b_o,
                           w1, b1, w2, b2, g1, be1, g2, be2)
    nc = _compiled()
    res = run_bass_kernel_spmd(nc, in_maps, core_ids=list(range(8)))
    return assemble_out(res.results)


# revision 71
# speedup vs baseline: 1.0017x; 1.0017x over previous
"""Trainium2 Bass kernel: transformer encoder layer (B=4, S=2048, D=1024, H=16, FF=4096).

Sharding (8 NeuronCores, no collectives): core c handles batch b=c//2 and
query-token half r=c%2 (1024 query rows). K/V are recomputed per core over the
batch's full 2048-token sequence (~12% duplicated FLOPs, zero communication).

Device layout: all activations are kept feature-on-partition ("transposed",
[d, tokens]) so every projection is matmul(lhsT=weight_natural, rhs=act_T).
Attention computes scores^T [k, q] per head (softmax denominators come from a
ones-column appended to V — row 64 of the AV accumulation), so no on-device
transposes are needed anywhere. The host passes x already transposed with the
core's query tokens first (attention is permutation-invariant over k; the
src_mask is all-ones), and pre-tiles every large operand so each DMA moves
>=4 KB contiguous per partition (1 KB-segment patterns run at ~65 GB/s vs
~358 GB/s line rate).

Numerics: matmul operands and the residual stream are bf16 (each residual
store rounds at ~0.2% rms — well inside the 2e-2 budget); V and the exp'd
scores are fp8e4m3 (their quantization noise averages out over ~2k positive
normalized attention weights); PSUM accumulation, LN statistics, and the
DMA'd output stay fp32. The exp is shifted by a constant (softmax is
shift-invariant, the ones-column denominator shifts identically) to keep
values under fp8's 448 max. LN sums read the bf16 stream directly (bf16 ones
column); sum-of-squares goes through an f32r Square on ScalarE; gamma/beta
are folded into the per-128-feature-chunk broadcast matmuls (g (x) rstd and
g (x) -mean*rstd + be (x) 1), so normalize is 2 VectorE ops per chunk.
Biases are exact (b_v folds into b_o on the host). Softmax skips the
max-subtraction.

Schedule: the PE queue executes in order, so emission order is the schedule.
Attention zippers score-groups of unit i+1 with AV chunks of unit i (a score
group stalls on exp draining its PSUM tile; AV chunks are always ready), and
attention overall is bounded by ScalarE exp throughput plus the hardware
activity throttle (the chip halves the PE clock for stretches of the
attention phase — PE+ACT dense concurrency; QKV and FFN run throttle-free).
w_o(chunk 0) interleaves with the second half of attention; w_o(chunk 1)
paces the LN1(0) stats+normalize; every later LN hides under the next
chunk's FFN matmuls, and the final LN2 sums chase FFN2 m-chunks so only the
stats chain and 8 normalize+DMA pairs remain at the end. ScalarE (idle
outside exp) applies matmul biases and LN squares; GpSimd does no compute
(its elementwise ops are ~3x slower and it cannot touch PSUM) — it only
hosts DMA queues for late-use prefetches (it also starts ~20 us late). QKV
runs m-major with token tiles innermost so each weight block is lhsT for 2
matmuls. w1/w2 stream per-chunk from host-pre-tiled layouts double-buffered
behind the FFN matmuls; the output is DMA'd per 128-feature chunk as LN2
finishes it, alternating the sync/scalar queues.
"""

import numpy as np
import ml_dtypes

import concourse.bass as bass
import concourse.tile as tile
from concourse import bacc
from concourse import mybir
from concourse.bass_utils import run_bass_kernel_spmd

P = 128
D = 1024          # d_model
S = 2048          # kv sequence length per core (one full batch)
TQ = 1024         # query tokens per core
H = 16            # heads
DK = 64           # head dim
FF = 4096         # ffn dim
DO = D // P       # 8  d_model chunks
KC = S // P       # 16 kv-token chunks
FO = FF // P      # 32 ffn chunks
NF = 512          # matmul free-dim tile
EPS = 1e-5

BF16 = mybir.dt.bfloat16
F32 = mybir.dt.float32
F32R = mybir.dt.float32r
FP8 = mybir.dt.float8e4
AF = mybir.ActivationFunctionType
ALU = mybir.AluOpType


def build():
    nc = bacc.Bacc("TRN2", target_bir_lowering=False, debug=False, num_devices=8)

    # all big operands arrive pre-tiled from the host so every DMA moves
    # >=4 KB contiguous per partition (1 KB-segment patterns measured ~65 GB/s)
    xt = nc.dram_tensor("xt", [P, DO, S], BF16, kind="ExternalInput").ap()
    wq = nc.dram_tensor("wq", [2, P, DO, NF], BF16, kind="ExternalInput").ap()
    wk = nc.dram_tensor("wk", [2, P, DO, NF], BF16, kind="ExternalInput").ap()
    wv = nc.dram_tensor("wv", [2, P, DO, NF], BF16, kind="ExternalInput").ap()
    wo = nc.dram_tensor("wo", [P, DO, D], BF16, kind="ExternalInput").ap()
    w1 = nc.dram_tensor("w1", [DO, P, DO, NF], BF16, kind="ExternalInput").ap()
    # w2 pre-tiled on host: [DO(m), 128(p=k-chunk lane), FO(kc), 128(m cols)]
    # so each 1 MB m-chunk is 8 KB contiguous per partition.
    w2p = nc.dram_tensor("w2p", [DO, P, FO, P], BF16, kind="ExternalInput").ap()
    bq = nc.dram_tensor("bq", [D], F32, kind="ExternalInput").ap()
    bk = nc.dram_tensor("bk", [D], F32, kind="ExternalInput").ap()
    bo = nc.dram_tensor("bo", [D], F32, kind="ExternalInput").ap()  # b_o + b_v@w_o
    b1v = nc.dram_tensor("b1", [FF], F32, kind="ExternalInput").ap()
    b2v = nc.dram_tensor("b2", [D], F32, kind="ExternalInput").ap()
    gb1 = nc.dram_tensor("gb1", [2, D], BF16, kind="ExternalInput").ap()
    gb2 = nc.dram_tensor("gb2", [2, D], BF16, kind="ExternalInput").ap()
    onesr = nc.dram_tensor("onesr", [P], F32R, kind="ExternalInput").ap()
    yt = nc.dram_tensor("yt", [D, TQ], F32, kind="ExternalOutput").ap()

    yt3 = yt.rearrange("(o p) t -> p o t", p=P)

    with tile.TileContext(nc) as tc:
        with (
            tc.tile_pool(name="persist", bufs=1) as persist,
            tc.tile_pool(name="lns", bufs=1) as lns,
            tc.tile_pool(name="work", bufs=2) as work,
            tc.tile_pool(name="psum", bufs=4, space="PSUM") as psum,
            tc.tile_pool(name="pssc", bufs=2, space="PSUM") as pssc,
        ):
            def load_vec(ap, n_chunks, name, eng):
                t = persist.tile([P, n_chunks], F32, tag=name)
                eng.dma_start(out=t[:], in_=ap.rearrange("(o p) -> p o", p=P))
                return t

            ones_col = persist.tile([P, 1], F32R, tag="ones_col")
            ones_col_bf = persist.tile([P, 1], BF16, tag="ones_col_bf")
            nc.vector.memset(ones_col_bf[:], 1.0)
            ones_row_dk = persist.tile([1, DK], BF16, tag="ones_row_dk")
            nc.vector.memset(ones_row_dk[:], 1.0)
            eps_sb = persist.tile([P, 1], F32, tag="eps")
            nc.vector.memset(eps_sb[:], EPS)
            eshift_sb = persist.tile([P, 1], F32, tag="eshift")
            nc.vector.memset(eshift_sb[:], -1.5)

            # ---------- layernorm pieces (feature dim on partitions) ----------
            def ln_sums(h_sl):
                """h_sl: [P, DO, NF] bf16. Returns (ps_s, ps_q) PSUM [1, NF]."""
                ps_s = psum.tile([P, NF], F32, tag="mm")
                ps_q = psum.tile([P, NF], F32, tag="mm")
                for o in range(DO):
                    nc.tensor.matmul(ps_s[0:1, :], lhsT=ones_col_bf,
                                     rhs=h_sl[:, o, :],
                                     start=(o == 0), stop=(o == DO - 1))
                    sq = work.tile([P, NF], F32R, tag="sq")
                    nc.scalar.activation(out=sq[:], in_=h_sl[:, o, :],
                                         func=AF.Square)
                    nc.tensor.matmul(ps_q[0:1, :], lhsT=ones_col, rhs=sq[:],
                                     start=(o == 0), stop=(o == DO - 1))
                return ps_s, ps_q

            def ln_stats(ps_s, ps_q, name):
                """Small per-token stats chain. Returns (rstd_bf [1,NF],
                nm [2,NF]) where nm row0 = -mean*rstd and row1 = ones."""
                mean = lns.tile([1, NF], F32, tag="ln_mean", name=f"mean_{name}")
                msq = lns.tile([1, NF], F32, tag="ln_msq", name=f"msq_{name}")
                nc.vector.tensor_scalar_mul(mean[:], ps_s[0:1, :], 1.0 / D)
                nc.vector.tensor_scalar_mul(msq[:], ps_q[0:1, :], 1.0 / D)
                var = lns.tile([1, NF], F32, tag="ln_var", name=f"var_{name}")
                nc.vector.tensor_mul(var[:], mean[:], mean[:])
                nc.vector.tensor_sub(var[:], msq[:], var[:])
                nc.scalar.activation(out=var[:], in_=var[:], func=AF.Sqrt,
                                     bias=eps_sb[0:1])
                rstd = lns.tile([1, NF], F32, tag="ln_rstd", name=f"rstd_{name}")
                nc.vector.reciprocal_approx_fast(out=rstd[:], in_=var[:])
                negms = msq  # msq dead; reuse
                nc.vector.tensor_mul(negms[:], mean[:], rstd[:])
                nc.vector.tensor_scalar_mul(negms[:], negms[:], -1.0)
                rstd_bf = lns.tile([1, NF], BF16, tag="ln_rstd_bf",
                                   name=f"rstdb_{name}")
                nc.vector.tensor_copy(rstd_bf[:], rstd[:])
                nm = lns.tile([2, NF], BF16, tag="ln_nm", name=f"nm_{name}")
                nc.vector.memset(nm[:], 1.0)  # row1 stays ones
                nc.vector.tensor_copy(nm[0:1, :], negms[:])
                return rstd_bf, nm

            def ln_bcast_o(o, rstd_bf, nm, gb_sb, name):
                """Per-o-chunk broadcast with gamma/beta folded in:
                row0 slot: g_o (x) rstd; row1 slot: g_o (x) negms + be_o (x) 1.
                bf16 K<=2 matmuls; rounding is ~0.2% rms — in budget."""
                ps = pssc.tile([P, 2, NF], F32, tag="sc2",
                               name=f"lnb_{name}_{o}")
                nc.tensor.matmul(ps[:, 0, :],
                                 lhsT=gb_sb[0:1, bass.ts(o, P)],
                                 rhs=rstd_bf[:], start=True, stop=True)
                nc.tensor.matmul(ps[:, 1, :],
                                 lhsT=gb_sb[0:2, bass.ts(o, P)],
                                 rhs=nm[:], start=True, stop=True)
                return ps

            def ln_norm_o(h_sl, o, ps, yt_out=None, dma_eng=None):
                if yt_out is None:
                    nc.vector.tensor_mul(h_sl[:, o, :], h_sl[:, o, :],
                                         ps[:, 0, :])
                    nc.vector.tensor_add(h_sl[:, o, :], h_sl[:, o, :],
                                         ps[:, 1, :])
                else:
                    yto = work.tile([P, NF], F32, tag="yto")
                    nc.vector.tensor_mul(yto[:], h_sl[:, o, :], ps[:, 0, :])
                    nc.vector.tensor_add(yto[:], yto[:], ps[:, 1, :])
                    dma_eng.dma_start(out=yt_out[:, o, :], in_=yto[:])

            with (
                tc.tile_pool(name="h1", bufs=1, side="right") as h1p,
                tc.tile_pool(name="wop", bufs=1, side="right") as wop,
            ):
                # h1f is preloaded with x (the residual); w_o accumulates
                # into it through a small ScalarE-biased temp
                h1f = h1p.tile([P, DO, TQ], BF16, tag="h1f")
                wo_sb = wop.tile([P, DO, D], BF16, tag="wo")

                def emit_wo_m(qn, m):
                    sl = bass.ts(qn, NF)
                    ps = psum.tile([P, NF], F32, tag="mm")
                    for kc in range(DO):
                        nc.tensor.matmul(
                            ps[:], lhsT=wo_sb[:, kc, bass.ts(m, P)],
                            rhs=ctx[:, kc, sl],
                            start=(kc == 0), stop=(kc == DO - 1))
                    wot = work.tile([P, NF], BF16, tag="wot")
                    nc.scalar.activation(
                        out=wot[:], in_=ps[:], func=AF.Identity,
                        bias=bo_sb[:, m:m + 1])
                    nc.vector.tensor_add(
                        h1f[:, m, sl], h1f[:, m, sl], wot[:])

                with tc.tile_pool(name="kqv", bufs=1) as kqvp:
                    kT = kqvp.tile([P, DO, S], BF16, tag="kT")
                    qT = kqvp.tile([P, DO, TQ], BF16, tag="qT")
                    # V and the exp'd scores are fp8e4m3: their quantization
                    # noise averages out across ~2k attention weights per
                    # token, and it enables DoubleRow AV matmuls (2 k-chunks
                    # per instruction)
                    vaug = kqvp.tile([P, KC, H * 65], FP8, tag="vaug")
                    vaug_h = vaug.rearrange("p t (h w) -> p t h w", w=65)
                    ctx = kqvp.tile([P, DO, TQ], BF16, tag="ctx")

                    # ---- phase 1: QKV projections ----
                    with (
                        tc.tile_pool(name="xtp", bufs=1) as xtp,
                        tc.tile_pool(name="wst", bufs=3) as wst,
                    ):
                        # startup: the gpsimd engine comes up ~20us late, so
                        # everything needed early rides the sync/scalar
                        # queues; gpsimd gets only far-future prefetches.
                        # Weights arrive in half-1024-column tiles so the
                        # m-major matmuls start after ~1 MB instead of 2 MB;
                        # xt streams per-128-feature chunk, matching the
                        # K-accumulation consumption order.
                        wk_h = [wst.tile([P, DO, NF], BF16, tag="w",
                                         name=f"wk_{h}") for h in range(2)]
                        nc.sync.dma_start(out=wk_h[0][:], in_=wk[0])
                        xt_sb = xtp.tile([P, DO, S], BF16, tag="xt")
                        # single-descriptor 1 MB quarters (per-partition
                        # contiguous), matching the K accumulation order
                        nc.scalar.dma_start(out=xt_sb[:, 0:2, :],
                                            in_=xt[:, 0:2, :])
                        nc.scalar.dma_start(out=xt_sb[:, 2:4, :],
                                            in_=xt[:, 2:4, :])
                        nc.sync.dma_start(out=xt_sb[:, 4:6, :],
                                          in_=xt[:, 4:6, :])
                        nc.sync.dma_start(out=wk_h[1][:], in_=wk[1])
                        nc.sync.dma_start(out=xt_sb[:, 6:8, :],
                                          in_=xt[:, 6:8, :])
                        for t in range(KC):
                            nc.vector.memset(vaug_h[:, t, :, 64:65], 1.0)
                        bk_sb = load_vec(bk, DO, "bk", nc.scalar)
                        wq_h = [wst.tile([P, DO, NF], BF16, tag="w",
                                         name=f"wq_{h}") for h in range(2)]
                        for h in range(2):
                            nc.sync.dma_start(out=wq_h[h][:], in_=wq[h])
                        bq_sb = load_vec(bq, DO, "bq", nc.sync)
                        # late-use loads on the slow-starting gpsimd queue
                        nc.gpsimd.dma_start(out=ones_col[:], in_=onesr[:, None])
                        bo_sb = load_vec(bo, DO, "bo", nc.gpsimd)
                        b2_sb = load_vec(b2v, DO, "b2", nc.gpsimd)
                        gb1_sb = persist.tile([2, D], BF16, tag="gb1")
                        nc.gpsimd.dma_start(out=gb1_sb[:], in_=gb1)
                        gb2_sb = persist.tile([2, D], BF16, tag="gb2")
                        nc.gpsimd.dma_start(out=gb2_sb[:], in_=gb2)
                        b1_sb = load_vec(b1v, FO, "b1", nc.gpsimd)
                        # prefetches for the post-attention pipeline (idle
                        # queues during QKV/attention); h1f starts as x
                        nc.gpsimd.dma_start(out=wo_sb[:], in_=wo)
                        nc.gpsimd.dma_start(out=h1f[:], in_=xt[:, :, 0:TQ])

                        # K^T/Q^T m-major, token tiles innermost: each weight
                        # column block is lhsT for 2 matmuls. K runs as two
                        # passes over token halves so the first pass only
                        # waits on the first 2 MB of xt.
                        def proj_T(w_h, out_sb, bias_sb, ns, tag):
                            for m in range(DO):
                                hh, mh = m // 4, m % 4
                                pss = [psum.tile([P, NF], F32, tag="mm",
                                                 name=f"pj_{tag}_{m}_{n}")
                                       for n in ns]
                                for kc in range(DO):
                                    for i, n in enumerate(ns):
                                        nc.tensor.matmul(
                                            pss[i][:],
                                            lhsT=w_h[hh][:, kc, bass.ts(mh, P)],
                                            rhs=xt_sb[:, kc, bass.ts(n, NF)],
                                            start=(kc == 0), stop=(kc == DO - 1))
                                for i, n in enumerate(ns):
                                    nc.vector.tensor_scalar(
                                        out=out_sb[:, m, bass.ts(n, NF)],
                                        in0=pss[i][:],
                                        scalar1=bias_sb[:, m:m + 1],
                                        scalar2=None, op0=ALU.add)

                        proj_T(wk_h, kT, bk_sb, [0, 1], "ka")
                        proj_T(wk_h, kT, bk_sb, [2, 3], "kb")
                        proj_T(wq_h, qT, bq_sb, [0, 1], "q")

                        # V in natural [token, d] layout, heads padded to 65
                        # cols (col 64 = ones -> softmax denominator); x token
                        # blocks are lhsT, reused across the 2 output tiles
                        wv_h = [wst.tile([P, DO, NF], BF16, tag="w",
                                         name=f"wv_{h}") for h in range(2)]
                        for h in range(2):
                            nc.sync.dma_start(out=wv_h[h][:], in_=wv[h])
                        for t in range(KC):
                            pss = [psum.tile([P, NF], F32, tag="mm",
                                             name=f"pv_{t}_{n}")
                                   for n in range(2)]
                            for kc in range(DO):
                                for n in range(2):
                                    nc.tensor.matmul(
                                        pss[n][:],
                                        lhsT=xt_sb[:, kc, bass.ts(t, P)],
                                        rhs=wv_h[n][:, kc, :],
                                        start=(kc == 0), stop=(kc == DO - 1))
                            for n in range(2):
                                nc.vector.tensor_copy(
                                    out=vaug_h[:, t, 8 * n:8 * n + 8, 0:64],
                                    in_=pss[n].rearrange("p (h w) -> p h w",
                                                         w=DK))

                    # ---- phase 2: attention ----
                    # software-pipelined: emit scores(pair i+1) before AV(pair
                    # i) so PE keeps running while ACT computes the exps. The
                    # w_o matmuls for query chunk 0 slot into the back half.
                    with tc.tile_pool(name="es", bufs=4) as esp:
                        KH = KC // 2  # kv chunks per half
                        units = [(qn, j, half) for qn in range(TQ // NF)
                                 for j in range(H // 2) for half in range(2)]

                        def alloc_es(i):
                            qn, j, half = units[i]
                            # per-head tiles: exp output stays contiguous
                            # (strided fp8 writes measurably slow ACT down)
                            return [esp.tile([P, KH, NF], FP8, tag="es",
                                             name=f"es_{qn}_{j}_{half}_{i2}")
                                    for i2 in range(2)]

                        def emit_scores_g(i, kc2, es01):
                            qn, j, half = units[i]
                            qsl = bass.ts(qn, NF)
                            # two 2-bank PSUM tiles (one per head), exp'd
                            # separately: keeps a second group in flight so
                            # the PE never waits a full exp latency
                            pss = [pssc.tile([P, 2, NF], F32, tag="sc2",
                                             name=f"sc2_{kc2}_{i2}")
                                   for i2 in range(2)]
                            for sub in range(2):
                                kc = half * KH + 2 * kc2 + sub
                                for idx in range(2):
                                    off = idx * DK
                                    nc.tensor.matmul(
                                        pss[idx][:, sub, :],
                                        lhsT=kT[off:off + DK, j, bass.ts(kc, P)],
                                        rhs=qT[off:off + DK, j, qsl],
                                        start=True, stop=True)
                            for idx in range(2):
                                # constant shift keeps exp() under fp8e4m3
                                # max (448); softmax is shift-invariant (the
                                # ones-column denominator shifts too)
                                nc.scalar.activation(
                                    out=es01[idx][:, 2 * kc2:2 * kc2 + 2, :],
                                    in_=pss[idx][:], func=AF.Exp,
                                    scale=0.125, bias=eshift_sb[:, 0:1])

                        def emit_av_part(i, part, es01, pcs):
                            qn, j, half = units[i]
                            for idx in range(2):
                                h = 2 * j + idx
                                for kl in (2 * part, 2 * part + 1):
                                    kc = half * KH + kl
                                    nc.tensor.matmul(
                                        pcs[idx][0:DK + 1, :],
                                        lhsT=vaug[:, kc, h * 65:(h + 1) * 65],
                                        rhs=es01[idx][:, kl, :],
                                        start=(kc == 0), stop=(kc == KC - 1))

                        def emit_av_tail(i, pcs):
                            qn, j, half = units[i]
                            if half == 0:
                                return
                            qsl = bass.ts(qn, NF)
                            for idx in range(2):
                                pc = pcs[idx]
                                den = lns.tile([1, NF], F32, tag="den")
                                nc.vector.tensor_copy(den[:], pc[DK:DK + 1, :])
                                rec = lns.tile([1, NF], F32, tag="rec")
                                nc.vector.reciprocal_approx_fast(
                                    out=rec[:], in_=den[:])
                                recb = esp.tile([1, NF], BF16, tag="recb")
                                nc.vector.tensor_copy(recb[:], rec[:])
                                ps_b = psum.tile([P, NF], F32, tag="mm")
                                nc.tensor.matmul(
                                    ps_b[0:DK, :], lhsT=ones_row_dk,
                                    rhs=recb[:], start=True, stop=True)
                                bc = esp.tile([DK, NF], BF16, tag="bc")
                                nc.vector.tensor_copy(bc[:], ps_b[0:DK, :])
                                nc.vector.tensor_mul(
                                    ctx[idx * DK:(idx + 1) * DK, j, qsl],
                                    pc[0:DK, :], bc[:])

                        # zipper: the PE queue is in-order, so score groups
                        # (which stall on exp draining PSUM) alternate with
                        # always-ready AV chunks of the previous unit
                        pcs = None
                        prev = alloc_es(0)
                        for g in range(KH // 2):
                            emit_scores_g(0, g, prev)
                        for i in range(len(units)):
                            qn, j, half = units[i]
                            cur = prev
                            if half == 0:
                                pcs = [psum.tile([P, NF], F32, tag="mm",
                                                 name=f"pc_{qn}_{j}_{i2}")
                                       for i2 in range(2)]
                            if i + 1 < len(units):
                                prev = alloc_es(i + 1)
                                for g in range(KH // 2):
                                    emit_scores_g(i + 1, g, prev)
                                    emit_av_part(i, g, cur, pcs)
                            else:
                                for g in range(KH // 2):
                                    emit_av_part(i, g, cur, pcs)
                            emit_av_tail(i, pcs)
                            # ctx for chunk 0 completes at unit 15: slide its
                            # w_o into the second half of attention
                            if i >= 16 and i % 2 == 1:
                                emit_wo_m(0, (i - 16) // 2)

                    # still inside the kqv pool (ctx is alive): LN1(0) sums,
                    # then w_o(1) m-chunks pace the LN1(0) stats + normalize
                    sl0, sl1 = bass.ts(0, NF), bass.ts(1, NF)
                    s0 = ln_sums(h1f[:, :, sl0])
                    emit_wo_m(1, 0)
                    emit_wo_m(1, 1)
                    st0 = ln_stats(*s0, name="ln1q0")
                    for o in range(DO):
                        ps = ln_bcast_o(o, *st0, gb1_sb, "l1q0")
                        ln_norm_o(h1f[:, :, sl0], o, ps)
                        if 2 <= o < DO:
                            emit_wo_m(1, o)
                    s1 = ln_sums(h1f[:, :, sl1])
                    st1 = ln_stats(*s1, name="ln1q1")

                # ---- phases 3+4, hand-pipelined across the 2 query chunks ----
                dq = [nc.sync, nc.scalar]
                with (
                    tc.tile_pool(name="aTp", bufs=2) as atp,
                    tc.tile_pool(name="wf1", bufs=2) as wf1p,
                    tc.tile_pool(name="wf2", bufs=2) as wf2p,
                ):
                    aT = [atp.tile([P, FO, NF], BF16, tag="aT", name=f"aT_{qn}")
                          for qn in range(2)]

                    def emit_ffn1_c(qn, c):
                        sl = bass.ts(qn, NF)
                        # 1 MB chunks with 1 KB contiguous rows; smaller
                        # slices DMA far below line rate
                        w1_sb = wf1p.tile([P, DO, NF], BF16, tag="w1")
                        nc.sync.dma_start(out=w1_sb[:], in_=w1[c])
                        for u in range(4):
                            mf = 4 * c + u
                            ps = psum.tile([P, NF], F32, tag="mm")
                            for kc in range(DO):
                                nc.tensor.matmul(
                                    ps[:], lhsT=w1_sb[:, kc, bass.ts(u, P)],
                                    rhs=h1f[:, kc, sl],
                                    start=(kc == 0), stop=(kc == DO - 1))
                            # bias+relu split between ScalarE and VectorE
                            if u % 2 == 0:
                                nc.scalar.activation(
                                    out=aT[qn][:, mf, :], in_=ps[:],
                                    func=AF.Relu, bias=b1_sb[:, mf:mf + 1])
                            else:
                                nc.vector.tensor_scalar(
                                    out=aT[qn][:, mf, :], in0=ps[:],
                                    scalar1=b1_sb[:, mf:mf + 1], scalar2=0.0,
                                    op0=ALU.add, op1=ALU.max)

                    def emit_ffn2_m(qn, m):
                        sl = bass.ts(qn, NF)
                        w2_sb = wf2p.tile([P, FO, P], BF16, tag="w2",
                                          name=f"w2_{qn}_{m}")
                        nc.gpsimd.dma_start(out=w2_sb[:], in_=w2p[m])
                        ps = psum.tile([P, NF], F32, tag="mm")
                        for kc in range(FO):
                            nc.tensor.matmul(
                                ps[:], lhsT=w2_sb[:, kc, :], rhs=aT[qn][:, kc, :],
                                start=(kc == 0), stop=(kc == FO - 1))
                        ep = work.tile([P, NF], BF16, tag="ep")
                        nc.scalar.activation(
                            out=ep[:], in_=ps[:], func=AF.Identity,
                            bias=b2_sb[:, m:m + 1])
                        nc.vector.tensor_add(h1f[:, m, sl], h1f[:, m, sl], ep[:])

                    # normalize both chunks; FFN1(0) fills in behind chunk 0
                    emit_ffn1_c(0, 0)
                    emit_ffn1_c(0, 1)
                    for o in range(DO):
                        ps = ln_bcast_o(o, *st1, gb1_sb, "l1q1")
                        ln_norm_o(h1f[:, :, sl1], o, ps)
                    for c in range(2, 8):
                        emit_ffn1_c(0, c)
                    # FFN2(0); then LN2(0) hides under FFN1(1)
                    for m in range(DO):
                        emit_ffn2_m(0, m)
                    s2 = ln_sums(h1f[:, :, sl0])
                    emit_ffn1_c(1, 0)
                    emit_ffn1_c(1, 1)
                    st2 = ln_stats(*s2, name="ln2q0")
                    for c in range(2, 8):
                        emit_ffn1_c(1, c)
                    for o in range(DO):
                        ps = ln_bcast_o(o, *st2, gb2_sb, "l2q0")
                        ln_norm_o(h1f[:, :, sl0], o, ps,
                                  yt_out=yt3[:, :, sl0], dma_eng=dq[o % 2])
                    # FFN2(1) + LN2(1) tail: the LN sums chase the FFN2
                    # m-chunks so only the stats chain remains at the end
                    ps_s3 = psum.tile([P, NF], F32, tag="mm")
                    ps_q3 = psum.tile([P, NF], F32, tag="mm")
                    for m in range(DO):
                        emit_ffn2_m(1, m)
                        nc.tensor.matmul(ps_s3[0:1, :], lhsT=ones_col_bf,
                                         rhs=h1f[:, m, sl1],
                                         start=(m == 0), stop=(m == DO - 1))
                        sq = work.tile([P, NF], F32R, tag="sq")
                        nc.scalar.activation(out=sq[:], in_=h1f[:, m, sl1],
                                             func=AF.Square)
                        nc.tensor.matmul(ps_q3[0:1, :], lhsT=ones_col, rhs=sq[:],
                                         start=(m == 0), stop=(m == DO - 1))
                    st3 = ln_stats(ps_s3, ps_q3, name="ln2q1")
                    for o in range(DO):
                        ps = ln_bcast_o(o, *st3, gb2_sb, "l2q1")
                        ln_norm_o(h1f[:, :, sl1], o, ps,
                                  yt_out=yt3[:, :, sl1], dma_eng=dq[o % 2])

    nc.compile()
    return nc


_CACHE = {}


def _compiled():
    if "nc" not in _CACHE:
        _CACHE["nc"] = build()
    return _CACHE["nc"]


def make_in_maps(x, w_q, b_q, w_k, b_k, w_v, b_v, w_o, b_o,
                 w1, b1, w2, b2, g1, be1, g2, be2):
    bf = ml_dtypes.bfloat16
    x = np.asarray(x, np.float32)
    f32 = lambda a: np.ascontiguousarray(np.asarray(a, np.float32))

    w_o32 = f32(w_o)
    w2_32 = f32(w2)

    # device-friendly tilings: every DMA reads >=4 KB contiguous/partition
    def tile_proj(w):  # [D, D] -> [2, 128, 8, 512]: w_t[h,p,o,n]=w[o*128+p, 512h+n]
        return np.ascontiguousarray(
            f32(w).reshape(DO, P, 2, NF).transpose(2, 1, 0, 3)).astype(bf)

    # w2p[m, p, kc, c] = w2[kc*128 + p, m*128 + c]
    w2p = np.ascontiguousarray(
        w2_32.reshape(FO, P, DO, P).transpose(2, 1, 0, 3)).astype(bf)
    # w1t[c, p, o, n] = w1[o*128 + p, c*512 + n]
    w1t = np.ascontiguousarray(
        f32(w1).reshape(DO, P, DO, NF).transpose(2, 1, 0, 3)).astype(bf)
    # wo_t[p, o, m] = wo[o*128 + p, m]
    wo_t = np.ascontiguousarray(w_o32.reshape(DO, P, D).transpose(1, 0, 2)
                                ).astype(bf)
    shared = {
        "wq": tile_proj(w_q), "wk": tile_proj(w_k), "wv": tile_proj(w_v),
        "wo": wo_t, "w1": w1t, "w2p": w2p,
        "bq": f32(b_q), "bk": f32(b_k),
        "bo": f32(b_o) + f32(b_v) @ w_o32,
        "b1": f32(b1), "b2": f32(b2),
        "gb1": np.stack([f32(g1), f32(be1)]).astype(bf),
        "gb2": np.stack([f32(g2), f32(be2)]).astype(bf),
        "onesr": np.ones((P,), np.float32),
    }
    in_maps = []
    for c in range(8):
        b, r = c // 2, c % 2
        xb = x[b]
        xc = np.concatenate([xb[r * TQ:(r + 1) * TQ], xb[(1 - r) * TQ:(2 - r) * TQ]],
                            axis=0)
        # xt_t[p, o, t] = x[t, o*128 + p]
        xt_t = np.ascontiguousarray(
            xc.T.reshape(DO, P, S).transpose(1, 0, 2)).astype(bf)
        m = dict(shared)
        m["xt"] = xt_t
        in_maps.append(m)
    return in_maps


def assemble_out(results):
    out = np.empty((4, 2048, 1024), np.float32)
    for c in range(8):
        b, r = c // 2, c % 2
        out[b, r * TQ:(r + 1) * TQ] = results[c]["yt"].T
    return out


def kernel(x, src_mask, w_q, b_q, w_k, b_k, w_v, b_v, w_o, b_o,
           w1, b1, w2, b2, g1, be1, g2, be2):
    in_maps = make_in_maps(x, w_q, b_q, w_k, b_k, w_v, b_v, w_o, b_o,
                           w1, b1, w2, b2, g1, be1, g2, be2)
    nc = _compiled()
    res = run_bass_kernel_spmd(nc, in_maps, core_ids=list(range(8)))
    return assemble_out(res.results)
